# revision 3
# baseline (speedup 1.0000x reference)
"""Distributed Trainium2 kernel for causal GQA attention with RoPE.

Model: B=2, S=2048, DM=2048, H=16 q-heads, HK=4 kv-heads, D=128.
Sharding over 8 NeuronCores: core c = (batch b=c//4, kv-head kh=c%4).
Each core computes its 4 q-heads / 1 kv-head of one batch end-to-end,
AllGathers attention outputs within its 4-core batch group, and applies
a column slice of Wo, producing out[b][:, kh*512:(kh+1)*512].
"""
import contextlib
import ctypes
import os
import sys
import types

for _p in ("/opt/trn_rl_repo", "/root/.axon_site/_ro/trn_rl_repo"):
    if os.path.isdir(_p) and _p not in sys.path:
        sys.path.insert(0, _p)

import numpy as np
import ml_dtypes

import concourse.bass as bass
import concourse.mybir as mybir
import concourse.tile as tile
from concourse import bacc
from concourse.bass import ts, ds
from concourse.bass_utils import run_bass_kernel_spmd
from concourse.masks import make_identity

BF16 = ml_dtypes.bfloat16
F32 = mybir.dt.float32
BF = mybir.dt.bfloat16

B, S, DM = 2, 2048, 2048
H, HK, D = 16, 4, 128
G = H // HK          # q heads per kv head (= heads per core)
THETA = 10000.0
N_CORES = 8
KT = DM // 128       # 16 K-tiles of the model dim
TOKB = S // 128      # 16 token blocks
TCH = S // 512       # 4 token chunks of 512
HD_CORE = G * D      # 512 output dims of q per core
NEG = -1.0e30

LAST_EXEC_TIME_NS = None
LAST_RESULTS = None


# ---------------------------------------------------------------- tracing
def _install_ntff_hook():
    """Make run_bass_kernel_spmd(trace=True) work in this container."""
    try:
        from antenv.axon_hooks import get_axon_ntff_profile_hook  # noqa: F401
        return True
    except ImportError:
        pass
    so_path = "/opt/axon/libaxon_pjrt.so"
    if not os.path.exists(so_path):
        return False
    lib = ctypes.CDLL(so_path)
    if not hasattr(lib, "axon_start_nrt_profile"):
        return False
    lib.axon_start_nrt_profile.argtypes = [ctypes.POINTER(ctypes.c_int64), ctypes.c_size_t]
    lib.axon_start_nrt_profile.restype = ctypes.c_int64
    lib.axon_stop_nrt_profile.argtypes = [ctypes.c_char_p]
    lib.axon_stop_nrt_profile.restype = ctypes.c_int64

    @contextlib.contextmanager
    def _hook(output_dir, device_ids):
        import jax
        jax.devices()
        if device_ids:
            ids = (ctypes.c_int64 * len(device_ids))(*device_ids)
            rc = lib.axon_start_nrt_profile(ids, len(device_ids))
        else:
            rc = lib.axon_start_nrt_profile(None, 0)
        if rc != 0:
            raise RuntimeError(f"axon_start_nrt_profile rc={rc}")
        try:
            yield
        finally:
            n = lib.axon_stop_nrt_profile(str(output_dir).encode())
            print(f"profile: {n} file(s) in {output_dir}", file=sys.stderr)

    mod = types.ModuleType("antenv.axon_hooks")
    holder = {"h": _hook}
    mod.set_axon_ntff_profile_hook = lambda h: holder.__setitem__("h", h)
    mod.get_axon_ntff_profile_hook = lambda: holder.get("h")
    sys.modules["antenv.axon_hooks"] = mod
    import antenv
    antenv.axon_hooks = mod
    import concourse.bass_utils as bu
    bu.upload_artifacts = lambda tmpdir: str(tmpdir)
    return True


# ---------------------------------------------------------------- graph
def build_nc():
    nc = bacc.Bacc("TRN2", target_bir_lowering=False, debug=False,
                   num_devices=N_CORES)

    xt = nc.dram_tensor("xt", [DM, S], BF, kind="ExternalInput").ap()
    wq = nc.dram_tensor("wq", [DM, HD_CORE], BF, kind="ExternalInput").ap()
    wk = nc.dram_tensor("wk", [DM, D], BF, kind="ExternalInput").ap()
    wv = nc.dram_tensor("wv", [DM, D], BF, kind="ExternalInput").ap()
    wo = nc.dram_tensor("wo", [DM, HD_CORE], BF, kind="ExternalInput").ap()
    cosq = nc.dram_tensor("cosq", [D, S], F32, kind="ExternalInput").ap()
    sinq = nc.dram_tensor("sinq", [D, S], F32, kind="ExternalInput").ap()
    cosk = nc.dram_tensor("cosk", [D, S], F32, kind="ExternalInput").ap()
    sink = nc.dram_tensor("sink", [D, S], F32, kind="ExternalInput").ap()
    out = nc.dram_tensor("out", [S, HD_CORE], F32, kind="ExternalOutput").ap()

    groups = [[0, 1, 2, 3], [4, 5, 6, 7]]

    with tile.TileContext(nc) as tc:
        with tc.tile_pool(name="const", bufs=1) as cpool, \
             tc.tile_pool(name="wts", bufs=1) as wpool, \
             tc.tile_pool(name="acts", bufs=1) as apool, \
             tc.tile_pool(name="xin", bufs=2) as xpool, \
             tc.tile_pool(name="work", bufs=2) as work, \
             tc.tile_pool(name="ework", bufs=2) as ework, \
             tc.tile_pool(name="etwork", bufs=3) as etwork, \
             tc.tile_pool(name="stats", bufs=3) as stats, \
             tc.tile_pool(name="psmm", bufs=4, space="PSUM") as ps_mm, \
             tc.tile_pool(name="pspv", bufs=2, space="PSUM") as ps_pv, \
             tc.tile_pool(name="pstr", bufs=2, space="PSUM") as ps_tr, \
             tc.tile_pool(name="dram", bufs=1, space="DRAM") as dpool:

            # ---------------- constants
            ident = cpool.tile([128, 128], BF, tag="ident", name="ident")
            make_identity(nc, ident[:])
            cmask = cpool.tile([128, 128], F32, tag="cmask", name="cmask")
            nc.gpsimd.memset(cmask[:], 0.0)
            nc.gpsimd.affine_select(
                out=cmask[:], in_=cmask[:],
                compare_op=mybir.AluOpType.is_ge, fill=NEG,
                base=0, pattern=[[-1, 128]], channel_multiplier=1)

            tbl = {}
            for name, src in (("cosq", cosq), ("sinq", sinq),
                              ("cosk", cosk), ("sink", sink)):
                t = cpool.tile([D, S], F32, tag=name)
                nc.sync.dma_start(out=t[:], in_=src[:])
                tbl[name] = t

            # ---------------- resident weights (already bf16 from host)
            wq_sb = wpool.tile([128, KT, HD_CORE], BF, tag="wq", name="wq")
            wk_sb = wpool.tile([128, KT, D], BF, tag="wk", name="wk")
            wv_sb = wpool.tile([128, KT, D], BF, tag="wv", name="wv")
            wo_sb = wpool.tile([128, KT, HD_CORE], BF, tag="wo", name="wo")
            for kt in range(KT):
                r = ds(128 * kt, 128)
                nc.sync.dma_start(out=wq_sb[:, kt, :], in_=wq[r, :])
                nc.sync.dma_start(out=wk_sb[:, kt, :], in_=wk[r, :])
                nc.sync.dma_start(out=wv_sb[:, kt, :], in_=wv[r, :])
                nc.sync.dma_start(out=wo_sb[:, kt, :], in_=wo[r, :])

            # ---------------- persistent activations
            qt_sb = [apool.tile([D, S], BF, tag=f"qt{h}", name=f"qt{h}") for h in range(G)]
            kt_sb = apool.tile([D, S], BF, tag="kt", name="kt")
            vtok_sb = apool.tile([128, TOKB, D], BF, tag="vtok", name="vtok")
            ot_sb = [apool.tile([D, S], BF, tag=f"ot{h}", name=f"ot{h}") for h in range(G)]

            # ---------------- projections + RoPE + v transpose
            # proj outputs, in emission order: k, v, q0..q3
            def rope_store(raw_ps, c, dst_slice, cos_t, sin_t):
                raw = work.tile([128, 512], F32, tag="qraw", name="qraw")
                nc.scalar.copy(raw[:], raw_ps[:])
                sh = work.tile([128, 512], F32, tag="sh", name="sh")
                nc.sync.dma_start(out=sh[0:64, :], in_=raw[64:128, :])
                nc.sync.dma_start(out=sh[64:128, :], in_=raw[0:64, :])
                t1 = work.tile([128, 512], F32, tag="t1", name="t1")
                nc.vector.tensor_mul(t1[:], sh[:], sin_t[:, ds(512 * c, 512)])
                t2 = work.tile([128, 512], F32, tag="t2", name="t2")
                nc.vector.tensor_mul(t2[:], raw[:], cos_t[:, ds(512 * c, 512)])
                nc.vector.tensor_add(dst_slice, t1[:], t2[:])

            for c in range(TCH):
                xc = xpool.tile([128, KT, 512], BF, tag="xc", name="xc")
                for kt in range(KT):
                    nc.sync.dma_start(
                        out=xc[:, kt, :],
                        in_=xt[ds(128 * kt, 128), ds(512 * c, 512)])
                # k
                ps = ps_mm.tile([128, 512], F32, tag="mm", name="mm")
                for kt in range(KT):
                    nc.tensor.matmul(ps[:], wk_sb[:, kt, :], xc[:, kt, :],
                                     start=(kt == 0), stop=(kt == KT - 1))
                rope_store(ps, c, kt_sb[:, ds(512 * c, 512)],
                           tbl["cosk"], tbl["sink"])
                # v (no rope; transpose to token-major)
                ps = ps_mm.tile([128, 512], F32, tag="mm", name="mm")
                for kt in range(KT):
                    nc.tensor.matmul(ps[:], wv_sb[:, kt, :], xc[:, kt, :],
                                     start=(kt == 0), stop=(kt == KT - 1))
                vst = work.tile([128, 512], BF, tag="vst", name="vst")
                nc.scalar.copy(vst[:], ps[:])
                trp = ps_tr.tile([128, 512], BF, tag="tr", name="tr")
                for j in range(4):
                    nc.tensor.transpose(trp[:, ts(j, 128)], vst[:, ts(j, 128)],
                                        ident[:])
                nc.vector.tensor_copy(out=vtok_sb[:, ds(4 * c, 4), :], in_=trp[:])
                # q heads
                for h in range(G):
                    ps = ps_mm.tile([128, 512], F32, tag="mm", name="mm")
                    for kt in range(KT):
                        nc.tensor.matmul(ps[:], wq_sb[:, kt, ts(h, 128)],
                                         xc[:, kt, :],
                                         start=(kt == 0), stop=(kt == KT - 1))
                    rope_store(ps, c, qt_sb[h][:, ds(512 * c, 512)],
                               tbl["cosq"], tbl["sinq"])

            # ---------------- attention per head, AllGather per head
            cc_out = []
            for h in range(G):
                for qi in range(TOKB):
                    kcols = 128 * (qi + 1)
                    nch = (kcols + 511) // 512
                    e_sb = ework.tile([128, S], BF, tag="E", name="E")
                    rs = stats.tile([128, 4], F32, tag="rs", name="rs")
                    for ci in range(nch):
                        w = min(512, kcols - 512 * ci)
                        sps = ps_mm.tile([128, 512], F32, tag="mm", name="mm")
                        nc.tensor.matmul(sps[:, :w], qt_sb[h][:, ts(qi, 128)],
                                         kt_sb[:, ds(512 * ci, w)],
                                         start=True, stop=True)
                        if ci == nch - 1:
                            # causal mask on the diagonal 128-block
                            off = kcols - 512 * ci - 128
                            nc.vector.tensor_add(
                                sps[:, ds(off, 128)], sps[:, ds(off, 128)],
                                cmask[:])
                        nc.scalar.activation(
                            out=e_sb[:, ds(512 * ci, w)], in_=sps[:, :w],
                            func=mybir.ActivationFunctionType.Exp,
                            accum_out=rs[:, ds(ci, 1)])
                    den = stats.tile([128, 1], F32, tag="den", name="den")
                    nc.vector.tensor_reduce(
                        out=den[:], in_=rs[:, :nch],
                        axis=mybir.AxisListType.X, op=mybir.AluOpType.add)
                    rec = stats.tile([128, 1], F32, tag="rec", name="rec")
                    nc.vector.reciprocal(rec[:], den[:])
                    # normalize row block of E in place
                    nc.vector.tensor_scalar_mul(
                        e_sb[:, :kcols], e_sb[:, :kcols], rec[:, 0:1])
                    # PV: oT[:, qi-block] += vtok.T @ E-tile.T over k blocks
                    ops = ps_pv.tile([128, 128], F32, tag="pv", name="pv")
                    for kb0 in range(0, qi + 1, 4):
                        nb = min(4, qi + 1 - kb0)
                        trp = ps_tr.tile([128, 512], BF, tag="tr", name="tr")
                        for j in range(nb):
                            nc.tensor.transpose(
                                trp[:, ts(j, 128)],
                                e_sb[:, ts(kb0 + j, 128)], ident[:])
                        et = etwork.tile([128, 512], BF, tag="et", name="et")
                        nc.vector.tensor_copy(out=et[:, :128 * nb],
                                              in_=trp[:, :128 * nb])
                        for j in range(nb):
                            kb = kb0 + j
                            nc.tensor.matmul(ops[:], vtok_sb[:, kb, :],
                                             et[:, ts(j, 128)],
                                             start=(kb == 0), stop=(kb == qi))
                    nc.scalar.copy(ot_sb[h][:, ts(qi, 128)], ops[:])
                # ship this head into the collective
                cin = dpool.tile([D, S], BF, tag=f"cin{h}", name=f"cin{h}")
                nc.sync.dma_start(out=cin[:], in_=ot_sb[h][:])
                cout = dpool.tile([4, D, S], BF, tag=f"cout{h}", name=f"cout{h}")
                nc.gpsimd.collective_compute(
                    "AllGather", mybir.AluOpType.bypass,
                    replica_groups=groups,
                    ins=[cin.opt()], outs=[cout.opt()])
                cc_out.append(cout)

            # ---------------- Wo (column slice) from gathered heads
            # gathered head (r, h) holds global kv-head r, local head h
            # => HD K-tile index kt = r*G + h
            order = [(h, r) for h in range(G) for r in range(4)]
            for tc_i in range(TCH):
                pso = [ps_mm.tile([128, 512], F32, tag="mm", name="mm") for _ in range(4)]
                for idx, (h, r) in enumerate(order):
                    og = etwork.tile([128, 512], BF, tag="og", name="og")
                    nc.sync.dma_start(out=og[:],
                                      in_=cc_out[h][r, :, ds(512 * tc_i, 512)])
                    kt = r * G + h
                    for tb in range(4):
                        nc.tensor.matmul(pso[tb][:], og[:, ts(tb, 128)],
                                         wo_sb[:, kt, :],
                                         start=(idx == 0), stop=(idx == 15))
                for tb in range(4):
                    ost = work.tile([128, 512], F32, tag="ost", name="ost")
                    nc.scalar.copy(ost[:], pso[tb][:])
                    nc.sync.dma_start(
                        out=out[ds(512 * tc_i + 128 * tb, 128), :],
                        in_=ost[:])

    nc.finalize()
    return nc


_NC_CACHE = {}


def _get_nc():
    if "nc" not in _NC_CACHE:
        _NC_CACHE["nc"] = build_nc()
    return _NC_CACHE["nc"]


def _rope_tables():
    inv = 1.0 / (THETA ** (np.arange(0, D, 2, dtype=np.float64) / D))  # [64]
    pos = np.arange(S, dtype=np.float64)
    fr = pos[:, None] * inv[None, :]                 # [S, 64]
    emb = np.concatenate([fr, fr], axis=1)           # [S, D]
    cos = np.cos(emb).T.astype(np.float32)           # [D, S]
    sin = np.sin(emb).T.astype(np.float32)
    sgn = np.where(np.arange(D) < D // 2, -1.0, 1.0).astype(np.float32)[:, None]
    scale = np.float32(D ** -0.5)
    return (cos * scale, sin * sgn * scale,          # q tables (pre-scaled)
            cos.copy(), sin * sgn)                   # k tables


def kernel(x, Wq, Wk, Wv, Wo):
    global LAST_EXEC_TIME_NS, LAST_RESULTS
    nc = _get_nc()
    cq, sq, ck, sk = _rope_tables()
    in_maps = []
    for c in range(N_CORES):
        b, kh = c // 4, c % 4
        in_maps.append({
            "xt": np.ascontiguousarray(x[b].T).astype(BF16),
            "wq": np.ascontiguousarray(Wq[:, kh * HD_CORE:(kh + 1) * HD_CORE]).astype(BF16),
            "wk": np.ascontiguousarray(Wk[:, kh * D:(kh + 1) * D]).astype(BF16),
            "wv": np.ascontiguousarray(Wv[:, kh * D:(kh + 1) * D]).astype(BF16),
            "wo": np.ascontiguousarray(Wo[:, kh * HD_CORE:(kh + 1) * HD_CORE]).astype(BF16),
            "cosq": cq, "sinq": sq, "cosk": ck, "sink": sk,
        })
    trace = os.environ.get("KERNEL_TRACE", "0") == "1" and _install_ntff_hook()
    res = run_bass_kernel_spmd(nc, in_maps, core_ids=list(range(N_CORES)),
                               trace=trace)
    LAST_EXEC_TIME_NS = res.exec_time_ns
    LAST_RESULTS = res
    out = np.empty((B, S, DM), dtype=np.float32)
    for c in range(N_CORES):
        b, kh = c // 4, c % 4
        out[b, :, kh * HD_CORE:(kh + 1) * HD_CORE] = res.results[c]["out"]
    return out
